# revision 20
# baseline (speedup 1.0000x reference)
"""RNN-T JointNet kernel for Trainium2, 8 NeuronCores.

Reference computation (B=4, T=256, U=64, D=640, H=640, V=1024):
    enc  = enc_out @ W_enc + b_enc          (B,T,H)
    pred = pred_out @ W_pred + b_pred       (B,U,H)
    joint = tanh(enc[:,:,None,:] + pred[:,None,:,:])
    logits = joint @ W_fc + b_fc            (B,T,U,V)
    out = log_softmax(logits, -1)

Sharding: the 1024 (b,t) rows are split into 8 chunks of 128; core i gets
batch b=i//2, t-rows (i%2)*128..+128, and computes its full (128,U,V) slab.

Per-core dataflow (H on partitions for the joint so the (t,u) broadcast-add
is a per-partition-scalar op and the joint matmul contraction dim is on
partitions):
    weights stream in via consolidated gpsimd cast-DMAs: W_enc/W_pred/enc/
    pred -> bf16, W_fc/b_fc -> fp8e4m3.
    encT/predT = PE-transpose (bf16);  epT_m / ppbT_m = bf16 matmuls.
    jw  = epT + ppbT[:,u]    (DVE/Pool tensor_scalar, bf16 4x mode)
    jwr = tanh(jw) -> fp8    (ACT, one inst per 8-u block)
    logits psum[128t,1024v] via fp8 DoubleRow matmuls: contraction pairs
      (k0,k1),(k2,k3),(k4,BIAS) -- the bias row rides in the free half of
      the last DoubleRow pair (lhsT slot = constant e0 block, rhs slot has
      b_fc in row 0), so b_fc costs zero extra PE time.
    exp_s = Exp(psum) -> discarded fp8 SBUF scratch, accum_out -> S[:,u]
    logS  = Ln(S) per u-pair (ACT; exp+ln share the natural_log_exp table,
      tanh is hoisted into 32-u superblocks -> 4 table loads total)
    out_s = psum - logS      (DVE tensor_scalar from PSUM, fp32)
    DMA out_s -> out[:, u:u+2, :]
"""

import numpy as np
from contextlib import ExitStack

import concourse.bass as bass
import concourse.bacc as bacc
import concourse.tile as tile
from concourse import mybir
from concourse.ap import AP
from concourse.bass_utils import run_bass_kernel_spmd
from concourse.masks import make_identity

F32 = mybir.dt.float32
BF16 = mybir.dt.bfloat16
F8 = mybir.dt.float8e4
DR = mybir.MatmulPerfMode.DoubleRow

B, T, U = 4, 256, 64
D, H, V = 640, 640, 1024
NCORES = 8
TC = (B * T) // NCORES        # 128 t-rows per core
KT = H // 128                 # 5 contraction tiles
UB = 8                        # u-block size for the add/tanh stage
SBU = 32                      # u's per superblock (tanh hoisted per sb)
NPAIR = 3                     # DoubleRow contraction pairs per v-bank
E0_OFF = SBU * 640            # start of the contiguous e0 region in jwr


def _build_module():
    nc = bacc.Bacc()
    enc = nc.declare_dram_parameter("enc", [TC, D], F32, isOutput=False)
    pred = nc.declare_dram_parameter("pred", [U, D], F32, isOutput=False)
    w_enc = nc.declare_dram_parameter("w_enc", [D, H], F32, isOutput=False)
    w_pred = nc.declare_dram_parameter("w_pred", [D, H], F32, isOutput=False)
    w_fc = nc.declare_dram_parameter("w_fc", [H, V], F32, isOutput=False)
    bc = nc.declare_dram_parameter("bc", [H], F32, isOutput=False)
    b_fc = nc.declare_dram_parameter("b_fc", [V], F32, isOutput=False)
    e0p = nc.declare_dram_parameter("e0p", [128, SBU * 128], F8, isOutput=False)
    out = nc.declare_dram_parameter("out", [TC, U, V], F32, isOutput=True)

    with ExitStack() as ctx:
        tc_ = ctx.enter_context(tile.TileContext(nc))
        _body(ctx, tc_, enc, pred, w_enc, w_pred, w_fc, bc, b_fc, e0p, out)
    # The act-table insertion pass picks the FIRST table containing each
    # activation's func. With the real tables that means Exp->exp_and_others
    # (id 0, no ln) and Ln->natural_log (id 5, no exp), bouncing 1283ns loads
    # on every ln<->exp transition (64 loads, 82us of ACT time). Hiding 'exp'
    # from the set-0 entry steers Exp onto natural_log_exp_and_others (id 6),
    # which satisfies both exp and ln, so the pass emits just one load per
    # tanh / exp+ln section boundary. The emitted ids are still valid for
    # the real tables (set 0 does contain tanh, set 6 exp+ln), so walrus
    # codegen and CoreSim agree.
    import concourse.bacc as bacc_mod
    from concourse.hw_specs import get_activation_tables as _real_tables

    def _doctored_tables(arch):
        t = dict(_real_tables(arch))
        first = next(iter(t))
        t[first] = t[first] - {mybir.ActivationFunctionType.Exp}
        return t

    saved = bacc_mod.get_activation_tables
    bacc_mod.get_activation_tables = _doctored_tables
    try:
        nc.compile()
    finally:
        bacc_mod.get_activation_tables = saved
    return nc


def _body(ctx, tc, enc, pred, w_enc, w_pred, w_fc, bc, b_fc, e0p, out):
    nc = tc.nc
    Tanh = mybir.ActivationFunctionType.Tanh
    Exp = mybir.ActivationFunctionType.Exp
    Ln = mybir.ActivationFunctionType.Ln

    singles = ctx.enter_context(tc.tile_pool(name="singles", bufs=1))

    ident = singles.tile([128, 128], BF16)
    make_identity(nc, ident)
    bc_sb = singles.tile([128, KT], F32)
    nc.sync.dma_start(out=bc_sb, in_=bc[:].rearrange("(k p) -> p k", p=128))

    # persistent main-loop tiles
    wfp = [singles.tile([128, 2 * V], F8, tag=f"wfp{p}", name=f"wfp{p}")
           for p in range(NPAIR)]
    encT = singles.tile([128, KT * 128], BF16)
    predT = singles.tile([128, KT * U], BF16)
    epT = singles.tile([128, KT * 128], BF16)
    ppbT = singles.tile([128, KT * U], F32)  # scalar operand must be fp32
    S_sb = singles.tile([128, U], F32)
    logS_sb = singles.tile([128, U], F32)
    # jwr layout per superblock: 32 contiguous 640-col tanh slots, then a
    # contiguous 32*128 e0 region (row0=1) for the (k4, bias) DoubleRow pair.
    jwr_t = [singles.tile([128, SBU * 768], F8, tag=f"jwr{s}", name=f"jwr{s}")
             for s in range(2)]

    # 4 jw bufs so sb1's adds (interleaved into sb0's pair stream on DVE/Pool)
    # only depend on sb0 tanh consumption.
    jpool = ctx.enter_context(tc.tile_pool(name="jw", bufs=4))

    # ---- weight / input loads (gpsimd SWDGE casts fp32 -> bf16/fp8) ----
    # DMA order matters: the DMA engines are a shared serial resource, so
    # enc+W_enc go first (they gate the projection pipeline).
    with tc.tile_pool(name="pro", bufs=1) as pro, \
         tc.tile_pool(name="pro_ps", bufs=2, space="PSUM") as pro_ps, \
         tc.tile_pool(name="pro_w", bufs=1) as pro_w:
        enc_sb = pro.tile([128, D], BF16, tag="enc_raw")
        nc.gpsimd.dma_start(out=enc_sb, in_=enc[:, :])
        # one consolidated cast-DMA per projection weight (k-tiles along free)
        wenc_sb = pro_w.tile([128, KT * H], BF16, tag="wenc", name="wenc")
        nc.gpsimd.dma_start(
            out=wenc_sb[:, :].rearrange("p (k h) -> p k h", h=H),
            in_=w_enc[:, :].rearrange("(k p) h -> p k h", p=128))
        pred_sb = pro.tile([64, D], BF16, tag="pred_raw")
        nc.gpsimd.dma_start(out=pred_sb, in_=pred[:, :])
        wpred_sb = pro_w.tile([128, KT * H], BF16, tag="wpred", name="wpred")
        nc.gpsimd.dma_start(
            out=wpred_sb[:, :].rearrange("p (k h) -> p k h", h=H),
            in_=w_pred[:, :].rearrange("(k p) h -> p k h", p=128))

        # W_fc pair tiles: wfp[p] cols = [vb0_even | vb0_odd | vb1_even |
        # vb1_odd] where even/odd = k-tiles 2p / 2p+1 (p=2 odd slot = bias).
        for p in range(2):
            nc.gpsimd.dma_start(
                out=wfp[p][:, :].rearrange("p (vb two v) -> p vb two v",
                                           two=2, v=512),
                in_=w_fc[2 * p * 128:(2 * p + 2) * 128, :].rearrange(
                    "(two p) (vb v) -> two p vb v", p=128,
                    v=512).transpose([1, 2, 0, 3]))
        wfp2v = wfp[2][:, :].rearrange("p (vb two v) -> p vb two v",
                                       two=2, v=512)
        nc.gpsimd.dma_start(
            out=wfp2v[:, :, 0, :],
            in_=w_fc[512:640, :].rearrange("p (vb v) -> p vb v", v=512))
        nc.vector.memset(wfp2v[:, :, 1, :], 0.0)
        for vb in range(2):
            nc.gpsimd.dma_start(
                out=wfp[2][0:1, vb * 1024 + 512:(vb + 1) * 1024],
                in_=b_fc[vb * 512:(vb + 1) * 512].rearrange(
                    "(o v) -> o v", o=1))

        # e0 regions of both jwr tiles (row0=1, rest 0); contiguous, no cast
        # -> plain HWDGE DMA on SP, keeping the gpsimd SWDGE queue short.
        for s in range(2):
            nc.sync.dma_start(
                out=jwr_t[s][:, E0_OFF:E0_OFF + SBU * 128], in_=e0p[:, :])

        # ---- transposes (PE) + projections (bf16 matmuls) ----
        for k in range(KT):
            ps = pro_ps.tile([128, 128], BF16, tag="tp")
            nc.tensor.transpose(ps, enc_sb[:, k * 128:(k + 1) * 128], ident)
            nc.vector.tensor_copy(encT[:, k * 128:(k + 1) * 128], ps)
        for k in range(KT):
            ps = pro_ps.tile([128, 64], BF16, tag="tpp")
            nc.tensor.transpose(ps[:, 0:64], pred_sb[:, k * 128:(k + 1) * 128],
                                ident[0:64, 0:64])
            nc.vector.tensor_copy(predT[:, k * 64:(k + 1) * 64], ps[:, 0:64])

        # sb0's broadcast-adds are interleaved k-major with the projection
        # m-loop so DVE works while PE projects the next m-tile and the first
        # tanh can start as soon as m=4 lands.
        sb0_jw = [jpool.tile([128, UB * 640], BF16, tag="jw", name=f"jw{i}")
                  for i in range(4)]
        for m in range(KT):
            ps = pro_ps.tile([128, TC], F32, tag="proj")
            for k in range(KT):
                nc.tensor.matmul(ps,
                                 wenc_sb[:, k * H + m * 128:k * H + (m + 1) * 128],
                                 encT[:, k * 128:(k + 1) * 128],
                                 start=(k == 0), stop=(k == KT - 1))
            nc.vector.tensor_copy(epT[:, m * 128:(m + 1) * 128], ps)
            ps = pro_ps.tile([128, U], F32, tag="projp")
            for k in range(KT):
                nc.tensor.matmul(ps,
                                 wpred_sb[:, k * H + m * 128:k * H + (m + 1) * 128],
                                 predT[:, k * 64:(k + 1) * 64],
                                 start=(k == 0), stop=(k == KT - 1))
            # fold b_enc+b_pred while copying out of PSUM
            nc.vector.tensor_scalar_add(ppbT[:, m * 64:(m + 1) * 64], ps,
                                        bc_sb[:, m:m + 1])
            for ub in range(4):
                for ul in range(UB):
                    u = ub * UB + ul
                    nc.vector.tensor_scalar_add(
                        sb0_jw[ub][:, ul * 640 + m * 128:ul * 640 + (m + 1) * 128],
                        epT[:, m * 128:(m + 1) * 128],
                        ppbT[:, m * 64 + u:m * 64 + u + 1])

    # ---- main loop ----
    # 4 psum tiles = all 8 banks; pair i+1's matmuls never wait on pair i's
    # subs, which keeps the exp stream dense on ACT.
    psum = ctx.enter_context(tc.tile_pool(name="psum", bufs=4, space="PSUM"))
    expp = ctx.enter_context(tc.tile_pool(name="exps", bufs=2))
    opool = ctx.enter_context(tc.tile_pool(name="outstage", bufs=8))

    def emit_adds(ub):
        """jw[u][k][t] = epT[k] + ppbT[k][:,u]; bf16 4x on DVE, k>=2 on Pool
        (these run concurrently with sb0's sub stream on DVE)."""
        jw = jpool.tile([128, UB * 640], BF16, tag="jw")
        for ul in range(UB):
            u = ub * UB + ul
            for k in range(KT):
                eng = nc.gpsimd if k >= 2 else nc.vector
                eng.tensor_scalar_add(
                    jw[:, ul * 640 + k * 128:ul * 640 + (k + 1) * 128],
                    epT[:, k * 128:(k + 1) * 128],
                    ppbT[:, k * 64 + u:k * 64 + u + 1])
        return jw

    def emit_tanh(ub, jw, jt):
        ubl = ub % 4
        nc.scalar.activation(
            jt[:, ubl * UB * 640:(ubl + 1) * UB * 640], jw[:, :], Tanh)

    def pair_lhs(jt, ul, p):
        if p < NPAIR - 1:
            return jt[:, ul * 640 + p * 256:ul * 640 + (p + 1) * 256].rearrange(
                "p (two f) -> p two f", two=2)
        # (k4, e0): two 128-col blocks at distance E0_OFF+ul*128 - (ul*640+512)
        base = jt[:, ul * 640 + 512:ul * 640 + 640]
        delta = (E0_OFF + ul * 128) - (ul * 640 + 512)
        return AP(tensor=base.tensor, offset=base.offset,
                  ap=[list(base.ap)[0], [delta, 2], [1, 128]])

    def emit_pair(pr, jt):
        """matmuls + exp for u-pair, then ln, subs, dma."""
        u0 = pr * 2
        pss = []
        for j in range(2):
            u = u0 + j
            ul = u % SBU
            ps = psum.tile([128, V], F32, tag="logits")
            for vb in range(2):
                for p in range(NPAIR):
                    rhs = wfp[p][:, vb * 1024:(vb + 1) * 1024].rearrange(
                        "p (two f) -> p two f", two=2)
                    nc.tensor.matmul(ps[:, vb * 512:(vb + 1) * 512],
                                     pair_lhs(jt, ul, p), rhs,
                                     start=(p == 0), stop=(p == NPAIR - 1),
                                     perf_mode=DR)
            ex = expp.tile([128, V], F8, tag="ex")
            nc.scalar.activation(ex, ps, Exp, accum_out=S_sb[:, u:u + 1])
            pss.append(ps)
        nc.scalar.activation(logS_sb[:, u0:u0 + 2], S_sb[:, u0:u0 + 2], Ln)
        # per-u output DMA: halves the staging granularity so the DMA
        # engines stay fed through the sb1 tanh sections (8 ob bufs of
        # backlog) and the tail drains sooner.
        for j in range(2):
            ob = opool.tile([128, V], F32, tag="ob")
            nc.vector.tensor_scalar_sub(ob, pss[j],
                                        logS_sb[:, u0 + j:u0 + j + 1])
            nc.sync.dma_start(out=out[:, u0 + j:u0 + j + 1, :], in_=ob)

    # superblock 0: adds (hoisted into the prologue) + tanh for ub0..3, then
    # 16 u-pairs; sb1's adds are interleaved into sb0's pair stream, and its
    # tanh is split into two 2-ub sections so the output-DMA backlog bridges
    # each ACT tanh block.
    sb1_jw = [None] * 4
    for ub in range(4):
        emit_tanh(ub, sb0_jw[ub], jwr_t[0])
    for pr in range(16):
        emit_pair(pr, jwr_t[0])
        if pr in (3, 6, 9, 12):
            ub = 4 + (pr - 3) // 3
            sb1_jw[ub - 4] = emit_adds(ub)
    for ub in (4, 5):
        emit_tanh(ub, sb1_jw[ub - 4], jwr_t[1])
    for pr in range(16, 24):
        emit_pair(pr, jwr_t[1])
    for ub in (6, 7):
        emit_tanh(ub, sb1_jw[ub - 4], jwr_t[1])
    for pr in range(24, 32):
        emit_pair(pr, jwr_t[1])


_NC_CACHE = None


def _get_module():
    global _NC_CACHE
    if _NC_CACHE is None:
        _NC_CACHE = _build_module()
    return _NC_CACHE


def _e0_pattern():
    import ml_dtypes
    e0 = np.zeros((128, SBU * 128), dtype=ml_dtypes.float8_e4m3)
    e0[0, :] = 1.0
    return e0


def kernel(enc_out, pred_out, W_enc, b_enc, W_pred, b_pred, W_fc, b_fc):
    nc = _get_module()
    enc_out = np.ascontiguousarray(enc_out, dtype=np.float32)
    pred_out = np.ascontiguousarray(pred_out, dtype=np.float32)
    shared = {
        "w_enc": np.ascontiguousarray(W_enc, dtype=np.float32),
        "w_pred": np.ascontiguousarray(W_pred, dtype=np.float32),
        "w_fc": np.ascontiguousarray(W_fc, dtype=np.float32),
        "bc": np.ascontiguousarray(b_enc + b_pred, dtype=np.float32),
        "b_fc": np.ascontiguousarray(b_fc, dtype=np.float32),
        "e0p": _e0_pattern(),
    }
    in_maps = []
    for i in range(NCORES):
        b = i // (T // TC)
        t0 = (i % (T // TC)) * TC
        in_maps.append({
            "enc": np.ascontiguousarray(enc_out[b, t0:t0 + TC, :]),
            "pred": np.ascontiguousarray(pred_out[b]),
            **shared,
        })
    res = run_bass_kernel_spmd(nc, in_maps, core_ids=list(range(NCORES)))
    full = np.empty((B, T, U, V), dtype=np.float32)
    for i in range(NCORES):
        b = i // (T // TC)
        t0 = (i % (T // TC)) * TC
        full[b, t0:t0 + TC] = res.results[i]["out"]
    return full


# revision 23
# speedup vs baseline: 1.0455x; 1.0455x over previous
"""RNN-T JointNet kernel for Trainium2, 8 NeuronCores.

Reference computation (B=4, T=256, U=64, D=640, H=640, V=1024):
    enc  = enc_out @ W_enc + b_enc          (B,T,H)
    pred = pred_out @ W_pred + b_pred       (B,U,H)
    joint = tanh(enc[:,:,None,:] + pred[:,None,:,:])
    logits = joint @ W_fc + b_fc            (B,T,U,V)
    out = log_softmax(logits, -1)

Sharding: the 1024 (b,t) rows are split into 8 chunks of 128; core i gets
batch b=i//2, t-rows (i%2)*128..+128, and computes its full (128,U,V) slab.

Per-core dataflow (H on partitions for the joint so the (t,u) broadcast-add
is a per-partition-scalar op and the joint matmul contraction dim is on
partitions):
    weights stream in via consolidated gpsimd cast-DMAs: W_enc/W_pred/enc/
    pred -> bf16, W_fc/b_fc -> fp8e4m3.
    encT/predT = PE-transpose (bf16);  epT_m / ppbT_m = bf16 matmuls.
    jw  = epT + ppbT[:,u]    (DVE/Pool tensor_scalar, bf16 4x mode)
    jwr = tanh(jw) -> fp8    (ACT, one inst per 8-u block)
    logits psum[128t,1024v] via fp8 DoubleRow matmuls: contraction pairs
      (k0,k1),(k2,k3),(k4,BIAS) -- the bias row rides in the free half of
      the last DoubleRow pair (lhsT slot = constant e0 block, rhs slot has
      b_fc in row 0), so b_fc costs zero extra PE time.
    exp_s = Exp(psum) -> discarded fp8 SBUF scratch, accum_out -> S[:,u]
    logS  = Ln(S) per u-pair (ACT; exp+ln share the natural_log_exp table,
      tanh is hoisted into 32-u superblocks -> 4 table loads total)
    out_s = psum - logS      (DVE tensor_scalar from PSUM, fp32)
    DMA out_s -> out[:, u:u+2, :]
"""

import numpy as np
from contextlib import ExitStack

import concourse.bass as bass
import concourse.bacc as bacc
import concourse.tile as tile
from concourse import mybir
from concourse.ap import AP
from concourse.bass_utils import run_bass_kernel_spmd
from concourse.masks import make_identity

F32 = mybir.dt.float32
BF16 = mybir.dt.bfloat16
F8 = mybir.dt.float8e4
DR = mybir.MatmulPerfMode.DoubleRow

B, T, U = 4, 256, 64
D, H, V = 640, 640, 1024
NCORES = 8
TC = (B * T) // NCORES        # 128 t-rows per core
KT = H // 128                 # 5 contraction tiles
UB = 8                        # u-block size for the add/tanh stage
SBU = 32                      # u's per superblock (tanh hoisted per sb)
NPAIR = 3                     # DoubleRow contraction pairs per v-bank
E0_OFF = SBU * 640            # start of the contiguous e0 region in jwr


def _build_module():
    nc = bacc.Bacc()
    enc = nc.declare_dram_parameter("enc", [TC, D], F32, isOutput=False)
    pred = nc.declare_dram_parameter("pred", [U, D], F32, isOutput=False)
    w_enc = nc.declare_dram_parameter("w_enc", [D, H], F32, isOutput=False)
    w_pred = nc.declare_dram_parameter("w_pred", [D, H], F32, isOutput=False)
    w_fc = nc.declare_dram_parameter("w_fc", [H, V], F32, isOutput=False)
    bc = nc.declare_dram_parameter("bc", [H], F32, isOutput=False)
    b_fc = nc.declare_dram_parameter("b_fc", [V], F32, isOutput=False)
    e0p = nc.declare_dram_parameter("e0p", [128, SBU * 128], F8, isOutput=False)
    out = nc.declare_dram_parameter("out", [TC, U, V], F32, isOutput=True)

    with ExitStack() as ctx:
        tc_ = ctx.enter_context(tile.TileContext(nc))
        _body(ctx, tc_, enc, pred, w_enc, w_pred, w_fc, bc, b_fc, e0p, out)
    # The act-table insertion pass picks the FIRST table containing each
    # activation's func. With the real tables that means Exp->exp_and_others
    # (id 0, no ln) and Ln->natural_log (id 5, no exp), bouncing 1283ns loads
    # on every ln<->exp transition (64 loads, 82us of ACT time). Hiding 'exp'
    # from the set-0 entry steers Exp onto natural_log_exp_and_others (id 6),
    # which satisfies both exp and ln, so the pass emits just one load per
    # tanh / exp+ln section boundary. The emitted ids are still valid for
    # the real tables (set 0 does contain tanh, set 6 exp+ln), so walrus
    # codegen and CoreSim agree.
    import concourse.bacc as bacc_mod
    from concourse.hw_specs import get_activation_tables as _real_tables

    def _doctored_tables(arch):
        t = dict(_real_tables(arch))
        first = next(iter(t))
        t[first] = t[first] - {mybir.ActivationFunctionType.Exp}
        return t

    saved = bacc_mod.get_activation_tables
    bacc_mod.get_activation_tables = _doctored_tables
    try:
        nc.compile()
    finally:
        bacc_mod.get_activation_tables = saved
    return nc


def _body(ctx, tc, enc, pred, w_enc, w_pred, w_fc, bc, b_fc, e0p, out):
    nc = tc.nc
    Tanh = mybir.ActivationFunctionType.Tanh
    Exp = mybir.ActivationFunctionType.Exp
    Ln = mybir.ActivationFunctionType.Ln

    singles = ctx.enter_context(tc.tile_pool(name="singles", bufs=1))

    ident = singles.tile([128, 128], BF16)
    make_identity(nc, ident)
    bc_sb = singles.tile([128, KT], F32)
    nc.sync.dma_start(out=bc_sb, in_=bc[:].rearrange("(k p) -> p k", p=128))

    # persistent main-loop tiles
    wfp = [singles.tile([128, 2 * V], F8, tag=f"wfp{p}", name=f"wfp{p}")
           for p in range(NPAIR)]
    encT = singles.tile([128, KT * 128], BF16)
    predT = singles.tile([128, KT * U], BF16)
    epT = singles.tile([128, KT * 128], BF16)
    ppbT = singles.tile([128, KT * U], F32)  # scalar operand must be fp32
    S_sb = singles.tile([128, U], F32)
    logS_sb = singles.tile([128, U], F32)
    # jwr layout per superblock: 32 contiguous 640-col tanh slots, then a
    # contiguous 32*128 e0 region (row0=1) for the (k4, bias) DoubleRow pair.
    jwr_t = [singles.tile([128, SBU * 768], F8, tag=f"jwr{s}", name=f"jwr{s}")
             for s in range(2)]

    # 4 jw bufs so sb1's adds (interleaved into sb0's pair stream on DVE/Pool)
    # only depend on sb0 tanh consumption.
    jpool = ctx.enter_context(tc.tile_pool(name="jw", bufs=4))

    # ---- weight / input loads (gpsimd SWDGE casts fp32 -> bf16/fp8) ----
    # DMA order matters: the DMA engines are a shared serial resource, so
    # enc+W_enc go first (they gate the projection pipeline).
    with tc.tile_pool(name="pro", bufs=1) as pro, \
         tc.tile_pool(name="pro_ps", bufs=2, space="PSUM") as pro_ps, \
         tc.tile_pool(name="pro_w", bufs=1) as pro_w:
        enc_sb = pro.tile([128, D], BF16, tag="enc_raw")
        nc.gpsimd.dma_start(out=enc_sb, in_=enc[:, :])
        # one consolidated cast-DMA per projection weight (k-tiles along free)
        wenc_sb = pro_w.tile([128, KT * H], BF16, tag="wenc", name="wenc")
        nc.gpsimd.dma_start(
            out=wenc_sb[:, :].rearrange("p (k h) -> p k h", h=H),
            in_=w_enc[:, :].rearrange("(k p) h -> p k h", p=128))
        pred_sb = pro.tile([64, D], BF16, tag="pred_raw")
        nc.gpsimd.dma_start(out=pred_sb, in_=pred[:, :])
        wpred_sb = pro_w.tile([128, KT * H], BF16, tag="wpred", name="wpred")
        nc.gpsimd.dma_start(
            out=wpred_sb[:, :].rearrange("p (k h) -> p k h", h=H),
            in_=w_pred[:, :].rearrange("(k p) h -> p k h", p=128))

        # W_fc pair tiles: wfp[p] cols = [vb0_even | vb0_odd | vb1_even |
        # vb1_odd] where even/odd = k-tiles 2p / 2p+1 (p=2 odd slot = bias).
        for p in range(2):
            nc.gpsimd.dma_start(
                out=wfp[p][:, :].rearrange("p (vb two v) -> p vb two v",
                                           two=2, v=512),
                in_=w_fc[2 * p * 128:(2 * p + 2) * 128, :].rearrange(
                    "(two p) (vb v) -> two p vb v", p=128,
                    v=512).transpose([1, 2, 0, 3]))
        wfp2v = wfp[2][:, :].rearrange("p (vb two v) -> p vb two v",
                                       two=2, v=512)
        nc.gpsimd.dma_start(
            out=wfp2v[:, :, 0, :],
            in_=w_fc[512:640, :].rearrange("p (vb v) -> p vb v", v=512))
        nc.vector.memset(wfp2v[:, :, 1, :], 0.0)
        for vb in range(2):
            nc.gpsimd.dma_start(
                out=wfp[2][0:1, vb * 1024 + 512:(vb + 1) * 1024],
                in_=b_fc[vb * 512:(vb + 1) * 512].rearrange(
                    "(o v) -> o v", o=1))

        # e0 regions of both jwr tiles (row0=1, rest 0); contiguous, no cast
        # -> plain HWDGE DMA on SP, keeping the gpsimd SWDGE queue short.
        for s in range(2):
            nc.sync.dma_start(
                out=jwr_t[s][:, E0_OFF:E0_OFF + SBU * 128], in_=e0p[:, :])

        # ---- transposes (PE) + projections (bf16 matmuls) ----
        for k in range(KT):
            ps = pro_ps.tile([128, 128], BF16, tag="tp")
            nc.tensor.transpose(ps, enc_sb[:, k * 128:(k + 1) * 128], ident)
            nc.vector.tensor_copy(encT[:, k * 128:(k + 1) * 128], ps)
        for k in range(KT):
            ps = pro_ps.tile([128, 64], BF16, tag="tpp")
            nc.tensor.transpose(ps[:, 0:64], pred_sb[:, k * 128:(k + 1) * 128],
                                ident[0:64, 0:64])
            nc.vector.tensor_copy(predT[:, k * 64:(k + 1) * 64], ps[:, 0:64])

        for m in range(KT):
            ps = pro_ps.tile([128, TC], F32, tag="proj")
            for k in range(KT):
                nc.tensor.matmul(ps,
                                 wenc_sb[:, k * H + m * 128:k * H + (m + 1) * 128],
                                 encT[:, k * 128:(k + 1) * 128],
                                 start=(k == 0), stop=(k == KT - 1))
            nc.vector.tensor_copy(epT[:, m * 128:(m + 1) * 128], ps)
            ps = pro_ps.tile([128, U], F32, tag="projp")
            for k in range(KT):
                nc.tensor.matmul(ps,
                                 wpred_sb[:, k * H + m * 128:k * H + (m + 1) * 128],
                                 predT[:, k * 64:(k + 1) * 64],
                                 start=(k == 0), stop=(k == KT - 1))
            # fold b_enc+b_pred while copying out of PSUM
            nc.vector.tensor_scalar_add(ppbT[:, m * 64:(m + 1) * 64], ps,
                                        bc_sb[:, m:m + 1])

    # ---- main loop ----
    # 4 psum tiles = all 8 banks; pair i+1's matmuls never wait on pair i's
    # subs, which keeps the exp stream dense on ACT.
    psum = ctx.enter_context(tc.tile_pool(name="psum", bufs=4, space="PSUM"))
    expp = ctx.enter_context(tc.tile_pool(name="exps", bufs=2))
    opool = ctx.enter_context(tc.tile_pool(name="outstage", bufs=5))

    def emit_adds(ub):
        """jw[u][k][t] = epT[k] + ppbT[k][:,u]; bf16 4x on DVE. Pool takes
        k>=2 for sb1's adds (they run concurrently with sb0's sub stream on
        DVE); sb0's adds stay off Pool, whose SWDGE DMA preps (~1us each)
        would gate the first tanh."""
        jw = jpool.tile([128, UB * 640], BF16, tag="jw")
        for ul in range(UB):
            u = ub * UB + ul
            for k in range(KT):
                eng = nc.gpsimd if (k >= 2 and ub >= 4) else nc.vector
                eng.tensor_scalar_add(
                    jw[:, ul * 640 + k * 128:ul * 640 + (k + 1) * 128],
                    epT[:, k * 128:(k + 1) * 128],
                    ppbT[:, k * 64 + u:k * 64 + u + 1])
        return jw

    def emit_tanh(ub, jw, jt):
        ubl = ub % 4
        nc.scalar.activation(
            jt[:, ubl * UB * 640:(ubl + 1) * UB * 640], jw[:, :], Tanh)

    def pair_lhs(jt, ul, p):
        if p < NPAIR - 1:
            return jt[:, ul * 640 + p * 256:ul * 640 + (p + 1) * 256].rearrange(
                "p (two f) -> p two f", two=2)
        # (k4, e0): two 128-col blocks at distance E0_OFF+ul*128 - (ul*640+512)
        base = jt[:, ul * 640 + 512:ul * 640 + 640]
        delta = (E0_OFF + ul * 128) - (ul * 640 + 512)
        return AP(tensor=base.tensor, offset=base.offset,
                  ap=[list(base.ap)[0], [delta, 2], [1, 128]])

    def emit_pair(pr, jt):
        """matmuls + exp for u-pair, then ln, subs, dma."""
        u0 = pr * 2
        pss = []
        for j in range(2):
            u = u0 + j
            ul = u % SBU
            ps = psum.tile([128, V], F32, tag="logits")
            for vb in range(2):
                for p in range(NPAIR):
                    rhs = wfp[p][:, vb * 1024:(vb + 1) * 1024].rearrange(
                        "p (two f) -> p two f", two=2)
                    nc.tensor.matmul(ps[:, vb * 512:(vb + 1) * 512],
                                     pair_lhs(jt, ul, p), rhs,
                                     start=(p == 0), stop=(p == NPAIR - 1),
                                     perf_mode=DR)
            ex = expp.tile([128, V], F8, tag="ex")
            nc.scalar.activation(ex, ps, Exp, accum_out=S_sb[:, u:u + 1])
            pss.append(ps)
        nc.scalar.activation(logS_sb[:, u0:u0 + 2], S_sb[:, u0:u0 + 2], Ln)
        ob = opool.tile([128, 2 * V], F32, tag="ob")
        for j in range(2):
            nc.vector.tensor_scalar_sub(ob[:, j * V:(j + 1) * V], pss[j],
                                        logS_sb[:, u0 + j:u0 + j + 1])
        nc.sync.dma_start(out=out[:, u0:u0 + 2, :], in_=ob)

    # superblock 0: adds+tanh for ub0..3, then 16 u-pairs; sb1's adds are
    # interleaved into sb0's pair stream, and its tanh is split into two
    # 2-ub sections so the output-DMA backlog bridges each ACT tanh block.
    sb1_jw = [None] * 4
    for ub in range(4):
        jw = emit_adds(ub)
        emit_tanh(ub, jw, jwr_t[0])
    for pr in range(16):
        emit_pair(pr, jwr_t[0])
        if pr in (3, 6, 9, 12):
            ub = 4 + (pr - 3) // 3
            sb1_jw[ub - 4] = emit_adds(ub)
    for ub in (4, 5):
        emit_tanh(ub, sb1_jw[ub - 4], jwr_t[1])
    for pr in range(16, 24):
        emit_pair(pr, jwr_t[1])
    for ub in (6, 7):
        emit_tanh(ub, sb1_jw[ub - 4], jwr_t[1])
    for pr in range(24, 32):
        emit_pair(pr, jwr_t[1])


_NC_CACHE = None


def _get_module():
    global _NC_CACHE
    if _NC_CACHE is None:
        _NC_CACHE = _build_module()
    return _NC_CACHE


def _e0_pattern():
    import ml_dtypes
    e0 = np.zeros((128, SBU * 128), dtype=ml_dtypes.float8_e4m3)
    e0[0, :] = 1.0
    return e0


def kernel(enc_out, pred_out, W_enc, b_enc, W_pred, b_pred, W_fc, b_fc):
    nc = _get_module()
    enc_out = np.ascontiguousarray(enc_out, dtype=np.float32)
    pred_out = np.ascontiguousarray(pred_out, dtype=np.float32)
    shared = {
        "w_enc": np.ascontiguousarray(W_enc, dtype=np.float32),
        "w_pred": np.ascontiguousarray(W_pred, dtype=np.float32),
        "w_fc": np.ascontiguousarray(W_fc, dtype=np.float32),
        "bc": np.ascontiguousarray(b_enc + b_pred, dtype=np.float32),
        "b_fc": np.ascontiguousarray(b_fc, dtype=np.float32),
        "e0p": _e0_pattern(),
    }
    in_maps = []
    for i in range(NCORES):
        b = i // (T // TC)
        t0 = (i % (T // TC)) * TC
        in_maps.append({
            "enc": np.ascontiguousarray(enc_out[b, t0:t0 + TC, :]),
            "pred": np.ascontiguousarray(pred_out[b]),
            **shared,
        })
    res = run_bass_kernel_spmd(nc, in_maps, core_ids=list(range(NCORES)))
    full = np.empty((B, T, U, V), dtype=np.float32)
    for i in range(NCORES):
        b = i // (T // TC)
        t0 = (i % (T // TC)) * TC
        full[b, t0:t0 + TC] = res.results[i]["out"]
    return full
